# revision 1
# baseline (speedup 1.0000x reference)
"""ConversationAwareSAGEConv on 8 Trainium2 NeuronCores (Bass/Tile).

Algorithm notes:
- The per-edge MLP message e = concat(h,ctx)[src] @ Wm + bm depends only on
  the src node, so LN(e)+relu is computed per NODE (N rows) instead of per
  edge (E rows): 10x less work.
- Nodes are relabeled by a degree-balanced permutation so every 128-node
  destination window receives a near-equal number of edges.
- Sharding: nodes are split into 8 contiguous shards (by new id). Each core
  computes msg for its shard (node-parallel matmul + LN), the msg table is
  all-gathered, then each core processes the edges pointing into its shard:
  indirect-DMA gathers of h[src] / msg[src] rows + one-hot segment-sum
  matmuls accumulate neigh_sum / conv_sum / cnt per 128-node window, and the
  gated fusion (SAGE linear + gate) is applied window-by-window.
"""
import sys

sys.path.insert(0, "/opt/trn_rl_repo")

import numpy as np
import ml_dtypes

import concourse.bass as bass
import concourse.bacc as bacc
import concourse.tile as tile
from concourse import mybir
import concourse.bass_utils as bass_utils
from concourse.masks import make_identity

BF16 = ml_dtypes.bfloat16
EPS = 1e-5
FP = mybir.dt.float32
BF = mybir.dt.bfloat16
I32 = mybir.dt.int32


class Cfg:
    def __init__(self, N, E, C=8, D=128):
        assert N % C == 0
        self.N, self.E, self.C, self.D = N, E, C, D
        self.NS = N // C                      # nodes per shard
        self.W = (self.NS + 127) // 128       # windows (=tiles) per core
        self.NSP = self.W * 128               # padded shard size
        self.lastw = self.NS - (self.W - 1) * 128  # rows in last window
        self.TW = 260                         # table row: h|msg|ones|pad
        self.cpw = None                       # chunks per window (set from data)


# ----------------------------------------------------------------- host prep

def _balanced_perm(dst, cfg):
    """new_id[old] such that every 128-node window gets ~equal in-edges."""
    import heapq
    N, C, W, NS = cfg.N, cfg.C, cfg.W, cfg.NS
    deg = np.bincount(dst, minlength=N)
    order = np.argsort(-deg, kind="stable")
    caps = np.full(C * W, 128, np.int64)
    caps[W - 1::W] = cfg.lastw
    heap = [(0, int(w)) for w in range(C * W)]
    heapq.heapify(heap)
    fill = np.zeros(C * W, np.int64)
    new_id = np.empty(N, np.int64)
    for old in order:
        d = int(deg[old])
        while True:
            load, w = heapq.heappop(heap)
            if fill[w] < caps[w]:
                break
        new_id[old] = (w // W) * NS + (w % W) * 128 + fill[w]
        fill[w] += 1
        if fill[w] < caps[w]:
            heapq.heappush(heap, (load + d, w))
    return new_id


def prep_inputs(h, ctx, src, dst, W_self, W_neigh, b_sage, Wm, bm, ln_g, ln_b,
                Wg, bg, cfg):
    N, C, NS, W, D = cfg.N, cfg.C, cfg.NS, cfg.W, cfg.D
    new_id = _balanced_perm(np.asarray(dst), cfg)
    inv = np.empty(N, np.int64)
    inv[new_id] = np.arange(N)

    X = np.concatenate([h, ctx], axis=1).astype(BF16)   # [N, 2D]
    Xp = X[inv]                                          # row n = node new-id n
    htab = Xp[:, :D].copy()                              # [N, D] bf16

    src_n = new_id[np.asarray(src)]
    dst_n = new_id[np.asarray(dst)]
    core = dst_n // NS
    win = (dst_n % NS) // 128
    slot = (dst_n % NS) % 128
    key = core * W + win
    o = np.argsort(key, kind="stable")
    key_s, src_s, slot_s = key[o], src_n[o], slot[o]
    bounds = np.searchsorted(key_s, np.arange(C * W + 1))
    gsizes = np.diff(bounds)
    cpw = max(1, int(np.max((gsizes + 127) // 128)))
    cfg.cpw = cpw

    src_idx = np.zeros((C, W, 128, cpw), np.int32)
    drel = np.full((C, W, 128, cpw), 300.0, np.float32)
    for g in range(C * W):
        a, b = bounds[g], bounds[g + 1]
        n = b - a
        if n == 0:
            continue
        buf_s = np.zeros(cpw * 128, np.int32)
        buf_d = np.full(cpw * 128, 300.0, np.float32)
        buf_s[:n] = src_s[a:b]
        buf_d[:n] = slot_s[a:b]
        c, w = g // W, g % W
        src_idx[c, w] = buf_s.reshape(cpw, 128).T
        drel[c, w] = buf_d.reshape(cpw, 128).T

    in_maps = []
    for c in range(C):
        xT = np.zeros((2 * D, cfg.NSP), BF16)
        xT[:, :NS] = Xp[c * NS:(c + 1) * NS].T
        in_maps.append(dict(
            xT=xT,
            hrows=htab[c * NS:(c + 1) * NS].copy(),
            src_idx=src_idx[c],
            drel=drel[c],
            Wm=Wm.astype(BF16),
            W_self=W_self.astype(BF16),
            W_neigh=W_neigh.astype(BF16),
            Wg=Wg.astype(BF16),
            bm=bm.reshape(1, D).astype(BF16),
            b_sage=b_sage.reshape(1, D).astype(BF16),
            bg=bg.reshape(1, D).astype(BF16),
            ln_g=np.broadcast_to(ln_g.astype(np.float32), (128, D)).copy(),
            ln_b=np.broadcast_to(ln_b.astype(np.float32), (128, D)).copy(),
        ))
    return in_maps, new_id


# --------------------------------------------------------------- device build

def build(cfg):
    N, C, NS, W, D, TW, cpw = cfg.N, cfg.C, cfg.NS, cfg.W, cfg.D, cfg.TW, cfg.cpw
    nc = bacc.Bacc("TRN2", target_bir_lowering=False, debug=False,
                   enable_asserts=False, num_devices=C)

    xT = nc.dram_tensor("xT", [2 * D, cfg.NSP], BF, kind="ExternalInput")
    hrows = nc.dram_tensor("hrows", [NS, D], BF, kind="ExternalInput")
    src_idx = nc.dram_tensor("src_idx", [W, 128, cpw], I32, kind="ExternalInput")
    drel = nc.dram_tensor("drel", [W, 128, cpw], FP, kind="ExternalInput")
    Wm = nc.dram_tensor("Wm", [2 * D, D], BF, kind="ExternalInput")
    W_self = nc.dram_tensor("W_self", [D, D], BF, kind="ExternalInput")
    W_neigh = nc.dram_tensor("W_neigh", [D, D], BF, kind="ExternalInput")
    Wg = nc.dram_tensor("Wg", [2 * D, D], BF, kind="ExternalInput")
    bm = nc.dram_tensor("bm", [1, D], BF, kind="ExternalInput")
    b_sage = nc.dram_tensor("b_sage", [1, D], BF, kind="ExternalInput")
    bg = nc.dram_tensor("bg", [1, D], BF, kind="ExternalInput")
    ln_g = nc.dram_tensor("ln_g", [128, D], FP, kind="ExternalInput")
    ln_b = nc.dram_tensor("ln_b", [128, D], FP, kind="ExternalInput")
    out = nc.dram_tensor("out", [NS, D], FP, kind="ExternalOutput")

    with tile.TileContext(nc) as tc:
        with (
            tc.tile_pool(name="const", bufs=1) as cp,
            tc.tile_pool(name="sb", bufs=3) as sb,
            tc.tile_pool(name="p3", bufs=2) as p3,
            tc.tile_pool(name="ps", bufs=2, space="PSUM") as ps,
            tc.tile_pool(name="dram", bufs=1, space="DRAM") as dr,
        ):
            # ---- constants
            iota = cp.tile([128, 128], FP)
            nc.gpsimd.iota(iota[:], pattern=[[1, 128]], base=0,
                           channel_multiplier=0,
                           allow_small_or_imprecise_dtypes=True)
            ident = cp.tile([128, 128], BF)
            make_identity(nc, ident[:])
            ones1 = cp.tile([1, 128], BF)
            nc.vector.memset(ones1[:], 1.0)
            wm_sb0 = cp.tile([D, D], BF)
            nc.sync.dma_start(wm_sb0[:], Wm[0:D, :])
            wm_sb1 = cp.tile([D, D], BF)
            nc.sync.dma_start(wm_sb1[:], Wm[D:2 * D, :])
            ws_sb = cp.tile([D, D], BF)
            nc.sync.dma_start(ws_sb[:], W_self[:])
            wn_sb = cp.tile([D, D], BF)
            nc.sync.dma_start(wn_sb[:], W_neigh[:])
            wg_sb0 = cp.tile([D, D], BF)
            nc.sync.dma_start(wg_sb0[:], Wg[0:D, :])
            wg_sb1 = cp.tile([D, D], BF)
            nc.sync.dma_start(wg_sb1[:], Wg[D:2 * D, :])
            bm_sb = cp.tile([1, D], BF)
            nc.sync.dma_start(bm_sb[:], bm[:])
            bs_sb = cp.tile([1, D], BF)
            nc.sync.dma_start(bs_sb[:], b_sage[:])
            bg_sb = cp.tile([1, D], BF)
            nc.sync.dma_start(bg_sb[:], bg[:])
            lng_sb = cp.tile([128, D], FP)
            nc.sync.dma_start(lng_sb[:], ln_g[:])
            lnb_sb = cp.tile([128, D], FP)
            nc.sync.dma_start(lnb_sb[:], ln_b[:])

            cc_in = dr.tile([NS, TW], BF)
            cc_out = dr.tile([N, TW], BF, addr_space="Shared")
            for t in range(W):
                rows = 128 if t < W - 1 else cfg.lastw
                hb = sb.tile([128, D], BF, tag="hb")
                nc.sync.dma_start(hb[:rows, :], hrows[t * 128:t * 128 + rows, :])
                nc.sync.dma_start(cc_in[t * 128:t * 128 + rows, 0:D],
                                  hb[:rows, :])

            # ---- phase 1: per-node msg = relu(LN(X @ Wm + bm)); fill cc_in
            for t in range(W):
                rows = 128 if t < W - 1 else cfg.lastw
                x0 = sb.tile([128, 128], BF, tag="x0")
                nc.sync.dma_start(x0[:], xT[0:128, t * 128:(t + 1) * 128])
                x1 = sb.tile([128, 128], BF, tag="x1")
                nc.sync.dma_start(x1[:], xT[128:256, t * 128:(t + 1) * 128])
                pe = ps.tile([128, D], FP, tag="mm")
                nc.tensor.matmul(out=pe[:], lhsT=x0[:], rhs=wm_sb0[:],
                                 start=True, stop=False)
                nc.tensor.matmul(out=pe[:], lhsT=x1[:], rhs=wm_sb1[:],
                                 start=False, stop=False)
                nc.tensor.matmul(out=pe[:], lhsT=ones1[:], rhs=bm_sb[:],
                                 start=False, stop=True)
                st6 = sb.tile([128, 6], FP, tag="st6")
                nc.vector.bn_stats(st6[:], pe[:])
                mv = sb.tile([128, 2], FP, tag="mv")
                nc.vector.bn_aggr(mv[:], st6[:])
                rinv = sb.tile([128, 1], FP, tag="rinv")
                nc.vector.tensor_scalar_add(rinv[:], mv[:, 1:2], EPS)
                nc.vector.reciprocal(rinv[:], rinv[:])
                s = sb.tile([128, 1], FP, tag="s")
                nc.scalar.sqrt(s[:], rinv[:])
                nmean = sb.tile([128, 1], FP, tag="nmean")
                nc.vector.tensor_scalar(out=nmean[:], in0=mv[:, 0:1],
                                        scalar1=s[:, :1], scalar2=-1.0,
                                        op0=mybir.AluOpType.mult,
                                        op1=mybir.AluOpType.mult)
                u = sb.tile([128, D], FP, tag="u")
                nc.scalar.activation(u[:], pe[:],
                                     mybir.ActivationFunctionType.Identity,
                                     bias=nmean[:, :1], scale=s[:, :1])
                nc.vector.tensor_tensor(out=u[:], in0=u[:], in1=lng_sb[:],
                                        op=mybir.AluOpType.mult)
                nc.vector.tensor_tensor(out=u[:], in0=u[:], in1=lnb_sb[:],
                                        op=mybir.AluOpType.add)
                msg = sb.tile([128, TW - D], BF, tag="msg")
                nc.vector.tensor_scalar_max(msg[:, 0:D], u[:], 0.0)
                nc.vector.memset(msg[:, D:D + 1], 1.0)
                nc.vector.memset(msg[:, D + 1:TW - D], 0.0)
                nc.sync.dma_start(cc_in[t * 128:t * 128 + rows, D:TW],
                                  msg[:rows, :])

            # ---- all-gather the msg table
            nc.gpsimd.collective_compute(
                "AllGather", mybir.AluOpType.bypass,
                replica_groups=[list(range(C))],
                ins=[cc_in.opt()], outs=[cc_out.opt()],
            )

            # ---- phase 2+3 per destination window
            for w in range(W):
                rows = 128 if w < W - 1 else cfg.lastw
                it = sb.tile([128, cpw], I32, tag="it")
                nc.sync.dma_start(it[:], src_idx[w])
                dre = sb.tile([128, cpw], FP, tag="dre")
                nc.sync.dma_start(dre[:], drel[w])
                ph = ps.tile([128, D], FP, tag="ph")
                pm = ps.tile([128, D + 1], FP, tag="pm")
                for j in range(cpw):
                    vt = sb.tile([128, TW], BF, tag="vt")
                    nc.gpsimd.indirect_dma_start(
                        out=vt[:], out_offset=None, in_=cc_out[:],
                        in_offset=bass.IndirectOffsetOnAxis(ap=it[:, j:j + 1],
                                                            axis=0))
                    A = sb.tile([128, 128], BF, tag="A")
                    nc.vector.tensor_scalar(out=A[:], in0=iota[:],
                                            scalar1=dre[:, j:j + 1],
                                            scalar2=None,
                                            op0=mybir.AluOpType.is_equal)
                    nc.tensor.matmul(out=ph[:], lhsT=A[:],
                                     rhs=vt[:, 0:D],
                                     start=(j == 0), stop=(j == cpw - 1))
                    nc.tensor.matmul(out=pm[:], lhsT=A[:],
                                     rhs=vt[:, D:2 * D + 1],
                                     start=(j == 0), stop=(j == cpw - 1))
                # phase 3: SAGE linear + gated fusion
                inv = p3.tile([128, 1], FP, tag="inv")
                nc.vector.tensor_scalar_max(inv[:], pm[:, D:D + 1], 1.0)
                nc.vector.reciprocal(inv[:], inv[:])
                nm_bf = p3.tile([128, D], BF, tag="nm_bf")
                nc.vector.tensor_scalar_mul(nm_bf[:], ph[:], inv[:, :1])
                ca_bf = p3.tile([128, D], BF, tag="ca_bf")
                nc.vector.tensor_scalar_mul(ca_bf[:], pm[:, 0:D], inv[:, :1])
                ptr = ps.tile([128, D], BF, tag="tr")
                nc.tensor.transpose(out=ptr[:], in_=nm_bf[:], identity=ident[:])
                nmT = p3.tile([128, D], BF, tag="nmT")
                nc.scalar.activation(nmT[:], ptr[:],
                                     mybir.ActivationFunctionType.Copy)
                hT = sb.tile([128, 128], BF, tag="x0")
                nc.sync.dma_start(hT[:], xT[0:128, w * 128:(w + 1) * 128])
                pstd = ps.tile([128, D], FP, tag="mm")
                nc.tensor.matmul(out=pstd[:], lhsT=hT[:], rhs=ws_sb[:],
                                 start=True, stop=False)
                nc.tensor.matmul(out=pstd[:], lhsT=nmT[:], rhs=wn_sb[:],
                                 start=False, stop=False)
                nc.tensor.matmul(out=pstd[:], lhsT=ones1[:], rhs=bs_sb[:],
                                 start=False, stop=True)
                std_bf = p3.tile([128, D], BF, tag="std_bf")
                nc.scalar.activation(std_bf[:], pstd[:],
                                     mybir.ActivationFunctionType.Copy)
                d = p3.tile([128, D], FP, tag="d")
                nc.vector.tensor_tensor(out=d[:], in0=pstd[:], in1=ca_bf[:],
                                        op=mybir.AluOpType.subtract)
                ptr2 = ps.tile([128, D], BF, tag="tr")
                nc.tensor.transpose(out=ptr2[:], in_=std_bf[:], identity=ident[:])
                stdT = p3.tile([128, D], BF, tag="stdT")
                nc.scalar.activation(stdT[:], ptr2[:],
                                     mybir.ActivationFunctionType.Copy)
                ptr3 = ps.tile([128, D], BF, tag="tr")
                nc.tensor.transpose(out=ptr3[:], in_=ca_bf[:], identity=ident[:])
                caT = p3.tile([128, D], BF, tag="caT")
                nc.scalar.activation(caT[:], ptr3[:],
                                     mybir.ActivationFunctionType.Copy)
                pg = ps.tile([128, D], FP, tag="mm")
                nc.tensor.matmul(out=pg[:], lhsT=stdT[:], rhs=wg_sb0[:],
                                 start=True, stop=False)
                nc.tensor.matmul(out=pg[:], lhsT=caT[:], rhs=wg_sb1[:],
                                 start=False, stop=False)
                nc.tensor.matmul(out=pg[:], lhsT=ones1[:], rhs=bg_sb[:],
                                 start=False, stop=True)
                g = p3.tile([128, D], FP, tag="g")
                nc.scalar.activation(g[:], pg[:],
                                     mybir.ActivationFunctionType.Sigmoid)
                o = p3.tile([128, D], FP, tag="o")
                nc.vector.tensor_tensor(out=o[:], in0=g[:], in1=d[:],
                                        op=mybir.AluOpType.mult)
                nc.vector.tensor_tensor(out=o[:], in0=o[:], in1=ca_bf[:],
                                        op=mybir.AluOpType.add)
                nc.sync.dma_start(out[w * 128:w * 128 + rows, :], o[:rows, :])

    nc.compile()
    return nc


# ----------------------------------------------------------------- entrypoint

def _install_ntff_shim():
    """Registers antenv.axon_hooks so trace=True can capture neuron profiles
    under axon (the agent image lacks the module)."""
    import contextlib, ctypes, types
    if "antenv.axon_hooks" in sys.modules:
        return
    try:
        lib = ctypes.CDLL("/opt/axon/libaxon_pjrt.so")
        assert hasattr(lib, "axon_start_nrt_profile")
    except Exception:
        return
    lib.axon_start_nrt_profile.argtypes = [ctypes.POINTER(ctypes.c_int64), ctypes.c_size_t]
    lib.axon_start_nrt_profile.restype = ctypes.c_int64
    lib.axon_stop_nrt_profile.argtypes = [ctypes.c_char_p]
    lib.axon_stop_nrt_profile.restype = ctypes.c_int64

    @contextlib.contextmanager
    def _hook(output_dir, device_ids):
        import jax
        jax.devices()
        if device_ids:
            ids = (ctypes.c_int64 * len(device_ids))(*device_ids)
            rc = lib.axon_start_nrt_profile(ids, len(device_ids))
        else:
            rc = lib.axon_start_nrt_profile(None, 0)
        if rc != 0:
            raise RuntimeError(f"axon_start_nrt_profile rc={rc}")
        try:
            yield
        finally:
            lib.axon_stop_nrt_profile(str(output_dir).encode())

    mod = types.ModuleType("antenv.axon_hooks")
    mod.get_axon_ntff_profile_hook = lambda: _hook
    mod.set_axon_ntff_profile_hook = lambda h: None
    sys.modules["antenv.axon_hooks"] = mod


def run(inputs, cfg, trace=False):
    if trace:
        _install_ntff_shim()
    in_maps, new_id = prep_inputs(**inputs, cfg=cfg)
    nc = build(cfg)
    res = bass_utils.run_bass_kernel_spmd(
        nc, in_maps, core_ids=list(range(cfg.C)), trace=trace)
    outp = np.concatenate([res.results[c]["out"] for c in range(cfg.C)], axis=0)
    result = outp[new_id].astype(np.float32)
    return result, res


def kernel(**inputs) -> np.ndarray:
    h = np.asarray(inputs["h"])
    cfg = Cfg(N=h.shape[0], E=np.asarray(inputs["src"]).shape[0])
    inputs = {k: np.asarray(v) for k, v in inputs.items()}
    result, _ = run(inputs, cfg)
    return result


if __name__ == "__main__":
    pass

